# revision 1
# baseline (speedup 1.0000x reference)
"""Trainium2 Bass kernel for a GQA LlamaAttention layer with a LUT-addressed
paged KV cache (B=2, S=1024, HID=4096, NH=32, NKV=8, HD=128, PAST=1024).

Sharding: tensor-parallel over heads across 8 cores. Core c owns query heads
4c..4c+3 and KV head c (column-parallel Wq/Wk/Wv, row-parallel Wo). Each core
produces a full [2048, 4096] partial of out @ Wo; the host sums the 8
partials (row-parallel unshard).

Device kernel (per core), all matmuls in float32r (full PE rate at N=512):
  phase A: qT/kT/vT = W.T @ hidden.T streamed over 32 k-tiles into wide
           multi-bank PSUM tiles; ACT evacuates PSUM->SBUF; RoPE applied by
           DVE as 6 wide ops per token chunk on the [head_dim, token]
           layout; V transposed back to [token, head_dim] via PE.
  phase B: per (batch, head): scores computed transposed sT[l, tok] =
           kT_tile.T @ qT (so the P*V matmul needs no transpose), exp via
           ACT with fused 1/sqrt(HD) scale (max-subtraction skipped: scores
           are O(10), safe in fp32), causal mask as one wide 0/1 multiply,
           softmax denominator via an in-place binary tree over the
           contiguous [128, 16, 512] probability tile + an all-ones 128x128
           matmul (which also broadcasts the result), normalization after
           the V matmul.
  phase C: out_partial[tok, :] += attn.T @ Wo over the 4 local heads.

The LUT structure is used on the host at shard time (values are read from
the actual input arrays and verified): the drain scatter is dead for the
attention output, and old cache rows are host-packed per KV head.
"""

import os
import sys

for _p in ("/root/.axon_site/_ro/trn_rl_repo", "/opt/trn_rl_repo"):
    if os.path.isdir(_p) and _p not in sys.path:
        sys.path.append(_p)

from contextlib import ExitStack

import numpy as np

import concourse.bass as bass
import concourse.mybir as mybir
import concourse.tile as tile
from concourse import bacc
from concourse.bass_utils import run_bass_kernel_spmd
from concourse.masks import make_identity

F32 = mybir.dt.float32
F32R = mybir.dt.float32r
AF = mybir.ActivationFunctionType

B, S, HID = 2, 1024, 4096
NH, NKV, HD = 32, 8, 128
PAST = 1024
L = PAST + S          # 2048 KV tokens per sequence
T = B * S             # 2048 flattened query tokens
D = HID
HQ = NH // NKV        # 4 query heads per core
NK = D // 128         # 32 contraction tiles for the projections
NJ = L // 128         # 16 l-tiles per batch
SCALE = float(HD) ** -0.5

N_CORES = 8


def r32(ap):
    return ap.bitcast(F32R)


def _build_program(loop_n: int = 1):
    nc = bacc.Bacc("TRN2", target_bir_lowering=False, debug=False)

    hidT_d = nc.dram_tensor("hidT", [D, T], F32, kind="ExternalInput").ap()
    wq_d = nc.dram_tensor("wq", [D, HQ * HD], F32, kind="ExternalInput").ap()
    wk_d = nc.dram_tensor("wk", [D, HD], F32, kind="ExternalInput").ap()
    wv_d = nc.dram_tensor("wv", [D, HD], F32, kind="ExternalInput").ap()
    wo_d = nc.dram_tensor("wo", [HQ * HD, D], F32, kind="ExternalInput").ap()
    cs_d = nc.dram_tensor("cs", [2, 128, T], F32, kind="ExternalInput").ap()
    koldT_d = nc.dram_tensor("koldT", [B, HD, PAST], F32, kind="ExternalInput").ap()
    vold_d = nc.dram_tensor("vold", [B, PAST, HD], F32, kind="ExternalInput").ap()
    bigc_d = nc.dram_tensor("bigc", [128, L], F32, kind="ExternalInput").ap()
    ones_d = nc.dram_tensor("ones", [128, 128], F32, kind="ExternalInput").ap()
    out_d = nc.dram_tensor("out", [T, D], F32, kind="ExternalOutput").ap()

    with ExitStack() as ctx:
        tc = ctx.enter_context(tile.TileContext(nc))

        def body(_iv=None):
            _emit(nc, tc, hidT_d, wq_d, wk_d, wv_d, wo_d, cs_d, koldT_d,
                  vold_d, bigc_d, ones_d, out_d)

        for _ in range(loop_n):
            body()

    nc.compile()
    return nc


def _emit(nc, tc, hidT_d, wq_d, wk_d, wv_d, wo_d, cs_d, koldT_d, vold_d,
          bigc_d, ones_d, out_d):
    with ExitStack() as ctx:
        pers = ctx.enter_context(tc.tile_pool(name="pers", bufs=1))

        # ---- persistent SBUF state ----
        # qTall[tc4][hd, h*512 + tok]  (tc4 = global 512-token chunk)
        qTall = [pers.tile([128, HQ * 512], F32R, tag=f"qTall{i}",
                           name=f"qTall{i}") for i in range(T // 512)]
        kT = [pers.tile([128, L], F32R, tag=f"kT{b}", name=f"kT{b}")
              for b in range(B)]
        # vnb[b][g][l % 128, ((l//128) % 4)*128 + hd]: 512 keys per group
        vnb = [[pers.tile([128, 512], F32R, tag=f"vnb{b}_{g}",
                          name=f"vnb{b}_{g}") for g in range(4)]
               for b in range(B)]

        # ---- phase A: projections + RoPE + V transpose ----
        with (
            tc.tile_pool(name="wq_sb", bufs=NK) as wq_pool,
            tc.tile_pool(name="wkv_sb", bufs=NK) as wkv_pool,
            tc.tile_pool(name="hid", bufs=3) as hid_pool,
            tc.tile_pool(name="csbuf", bufs=1) as cs_pool,
            tc.tile_pool(name="ident", bufs=1) as id_pool,
            tc.tile_pool(name="qraw", bufs=1) as qraw_pool,
            tc.tile_pool(name="kvraw", bufs=1) as kvraw_pool,
            tc.tile_pool(name="ropetmp", bufs=1) as rt_pool,
            tc.tile_pool(name="q_ps", bufs=1, space="PSUM") as qps_pool,
            tc.tile_pool(name="kv_ps", bufs=1, space="PSUM") as kvps_pool,
            tc.tile_pool(name="tr_ps", bufs=2, space="PSUM") as trps_pool,
        ):
            cos_sb = cs_pool.tile([128, T], F32, tag="cosf", name="cosf")
            sin_sb = cs_pool.tile([128, T], F32, tag="sinf", name="sinf")
            ident = id_pool.tile([128, 128], F32)
            make_identity(nc, ident[:])
            wq_sb, wkv_sb = [], []

            def rope6(raw, dest, tok0, nh):
                """RoPE [128, nh*512] raw (heads side by side, same tokens
                tok0..tok0+512) into dest; 5 wide DVE ops. Both inputs of
                every TensorTensor share a base partition (HW constraint)."""
                ts_ = slice(tok0, tok0 + 512)
                def bc(ap):
                    return ap.unsqueeze(1).broadcast_to(
                        [ap.shape[0], nh, 512])
                r3 = lambda ap: ap.rearrange("p (n f) -> p n f", n=nh)
                tr = rt_pool.tile([128, HQ * 512], F32, tag="tr",
                                  name="tr")[:, 0:nh * 512]
                nc.vector.tensor_mul(r3(tr[0:64, :]), r3(raw[64:128, :]),
                                     bc(sin_sb[64:128, ts_]))
                nc.vector.tensor_mul(r3(tr[64:128, :]), r3(raw[0:64, :]),
                                     bc(sin_sb[0:64, ts_]))
                nc.vector.tensor_mul(r3(dest[:, :]), r3(raw[:, :]),
                                     bc(cos_sb[:, ts_]))
                nc.vector.tensor_sub(dest[0:64, :], dest[0:64, :],
                                     tr[0:64, :])
                nc.vector.tensor_add(dest[64:128, :], dest[64:128, :],
                                     tr[64:128, :])

            for tc4 in range(T // 512):
                b = tc4 // 2
                qps = qps_pool.tile([128, HQ * 512], F32, name="qps")
                kvps = kvps_pool.tile([128, 1024], F32, name="kvps")
                for k in range(NK):
                    if tc4 == 0:
                        t = wq_pool.tile([128, HQ * HD], F32R, name="wq_sb")
                        nc.sync.dma_start(
                            t[:], r32(wq_d[k * 128:(k + 1) * 128, :]))
                        wq_sb.append(t)
                        t = wkv_pool.tile([128, 2 * HD], F32R, name="wkv_sb")
                        nc.sync.dma_start(
                            t[:, 0:HD], r32(wk_d[k * 128:(k + 1) * 128, :]))
                        nc.sync.dma_start(
                            t[:, HD:2 * HD],
                            r32(wv_d[k * 128:(k + 1) * 128, :]))
                        wkv_sb.append(t)
                    ht = hid_pool.tile([128, 512], F32R)
                    nc.sync.dma_start(
                        ht[:],
                        r32(hidT_d[k * 128:(k + 1) * 128,
                                   tc4 * 512:(tc4 + 1) * 512]))
                    st, sp = k == 0, k == NK - 1
                    for h in range(HQ):
                        nc.tensor.matmul(
                            qps[:, h * 512:(h + 1) * 512],
                            wq_sb[k][:, h * 128:(h + 1) * 128],
                            ht[:], start=st, stop=sp)
                    nc.tensor.matmul(kvps[:, 0:512], wkv_sb[k][:, 0:HD],
                                     ht[:], start=st, stop=sp)
                    nc.tensor.matmul(kvps[:, 512:1024],
                                     wkv_sb[k][:, HD:2 * HD],
                                     ht[:], start=st, stop=sp)
                if tc4 == 0:
                    nc.sync.dma_start(cos_sb[:], cs_d[0])
                    nc.sync.dma_start(sin_sb[:], cs_d[1])
                if tc4 == 1:
                    # old KV pages: overlap their DMA with phase-A compute
                    for bb in range(B):
                        nc.sync.dma_start(kT[bb][:, 0:PAST], r32(koldT_d[bb]))
                        for g in range(2):
                            src = vold_d[bb, g * 512:(g + 1) * 512, :]
                            nc.sync.dma_start(
                                vnb[bb][g][:].rearrange("p (j h) -> p j h", j=4),
                                r32(src.rearrange("(j p) h -> p j h", p=128)))
                # evacuate PSUM on ACT (copy), rope on DVE in SBUF
                qraw = qraw_pool.tile([128, HQ * 512], F32, name="qraw")
                nc.scalar.copy(qraw[:], qps[:])
                kvraw = kvraw_pool.tile([128, 1024], F32, name="kvraw")
                nc.scalar.copy(kvraw[:], kvps[:])
                rope6(qraw, qTall[tc4][:], tc4 * 512, HQ)
                koff = PAST + (tc4 % 2) * 512
                rope6(kvraw[:, 0:512], kT[b][:, koff:koff + 512],
                      tc4 * 512, 1)
                # V: 4 PE transposes into one [128, 512] psum, one DVE copy
                g = 2 + tc4 % 2
                tp = trps_pool.tile([128, 512], F32, name="tp")
                for jj in range(4):
                    nc.tensor.transpose(
                        tp[:, jj * 128:(jj + 1) * 128],
                        kvraw[:, 512 + jj * 128:512 + (jj + 1) * 128],
                        ident[:])
                nc.vector.tensor_copy(vnb[b][g][:], tp[:])

        # ---- phase B: attention per (batch, local head) ----
        outT_pool = ctx.enter_context(tc.tile_pool(name="outT_sb", bufs=1))
        # outT[b][hd, h*1024 + tok]: o_proj lhsT layout
        outT = [outT_pool.tile([128, HQ * S], F32R, tag=f"outT{b}",
                               name=f"outT{b}") for b in range(B)]
        with (
            tc.tile_pool(name="bigc", bufs=1) as bc_pool,
            tc.tile_pool(name="onesp", bufs=1) as ones_pool,
            tc.tile_pool(name="psT", bufs=14) as psT_pool,
            tc.tile_pool(name="recip", bufs=2) as rec_pool,
            tc.tile_pool(name="sc_ps", bufs=2, space="PSUM") as scps_pool,
            tc.tile_pool(name="av_ps", bufs=1, space="PSUM") as avps_pool,
            tc.tile_pool(name="dn_ps", bufs=1, space="PSUM") as dnps_pool,
        ):
            bigC = bc_pool.tile([128, L], F32)
            nc.sync.dma_start(bigC[:], bigc_d[:])
            ones = ones_pool.tile([128, 128], F32R)
            nc.sync.dma_start(ones[:], r32(ones_d[:]))

            for b in range(B):
                for h in range(HQ):
                    av = avps_pool.tile([128, 1024], F32, name="av")
                    dn = dnps_pool.tile([128, 1024], F32, name="dn")
                    for t2 in range(2):
                        # tile j is fully causal-masked when
                        # 128*j > PAST + 512*t2 + 511
                        nj_t = 12 if t2 == 0 else NJ
                        qrhs = qTall[b * 2 + t2][:, h * 512:(h + 1) * 512]
                        for jp in range(nj_t // 2):  # j-pairs share one psum
                            sc = scps_pool.tile([128, 1024], F32, name="sc")
                            for jo in range(2):
                                j = jp * 2 + jo
                                nc.tensor.matmul(
                                    sc[:, jo * 512:(jo + 1) * 512],
                                    kT[b][:, j * 128:(j + 1) * 128],
                                    qrhs, start=True, stop=True)
                            pst = psT_pool.tile([128, 2, 512], F32R,
                                                name="pst")
                            nc.scalar.activation(
                                pst[:].rearrange("p a b -> p (a b)"),
                                sc[:], AF.Exp, scale=SCALE)
                            if jp >= 4:
                                # causal mask, both tiles of the pair: bigC
                                # cols for tile j=8+j' start at
                                # PAST - 128*j' + t2*512 -> j-dim stride -128
                                moff = PAST - 128 * (jp * 2 - 8) + t2 * 512
                                mask_ap = bass.AP(
                                    tensor=bigC.tensor,
                                    offset=bigC.offset + moff,
                                    ap=[list(bigC.ap[0]), [-128, 2],
                                        [1, 512]])
                                nc.vector.tensor_mul(pst[:], pst[:], mask_ap)
                            # AV + denominator accumulate over the l-tiles
                            # (the all-ones lhsT of dn reduces over l AND
                            # broadcasts the sum to all output partitions)
                            for jo in range(2):
                                j = jp * 2 + jo
                                nc.tensor.matmul(
                                    av[:, t2 * 512:(t2 + 1) * 512],
                                    vnb[b][j // 4][:, (j % 4) * 128:
                                                   (j % 4 + 1) * 128],
                                    pst[:, jo, :],
                                    start=(j == 0), stop=(j == nj_t - 1))
                                nc.tensor.matmul(
                                    dn[:, t2 * 512:(t2 + 1) * 512], ones[:],
                                    pst[:, jo, :],
                                    start=(j == 0), stop=(j == nj_t - 1))
                    rec = rec_pool.tile([128, 1024], F32, name="rec")
                    nc.vector.reciprocal(rec[:], dn[:])
                    nc.vector.tensor_mul(
                        outT[b][:, h * S:(h + 1) * S], av[:], rec[:])

        # ---- phase C: o_proj partial ----
        with (
            tc.tile_pool(name="wo_sb", bufs=HQ) as wo_pool,
            tc.tile_pool(name="ostage", bufs=4) as ost_pool,
            tc.tile_pool(name="op_ps", bufs=4, space="PSUM") as opps_pool,
        ):
            wo_sb = []
            for h in range(HQ):
                t = wo_pool.tile([128, D], F32R, name="wo_sb")
                nc.sync.dma_start(t[:], r32(wo_d[h * 128:(h + 1) * 128, :]))
                wo_sb.append(t)
            for tt in range(T // 128):
                b, tloc = tt // (S // 128), tt % (S // 128)
                for n8 in range(D // 512):
                    op = opps_pool.tile([128, 512], F32, name="op")
                    for h in range(HQ):
                        nc.tensor.matmul(
                            op[:],
                            outT[b][:, h * S + tloc * 128:
                                    h * S + (tloc + 1) * 128],
                            wo_sb[h][:, n8 * 512:(n8 + 1) * 512],
                            start=(h == 0), stop=(h == HQ - 1))
                    ost = ost_pool.tile([128, 512], F32, name="ost")
                    nc.scalar.copy(ost[:], op[:])
                    nc.sync.dma_start(
                        out_d[tt * 128:(tt + 1) * 128,
                              n8 * 512:(n8 + 1) * 512],
                        ost[:])


_NC_CACHE = {}


def _get_program(loop_n: int = 1):
    if loop_n not in _NC_CACHE:
        _NC_CACHE[loop_n] = _build_program(loop_n)
    return _NC_CACHE[loop_n]


def make_in_maps(hidden_states, kv_cache, rope_cache, Wq, Wk, Wv, Wo,
                 position_offsets, kv_drain_addr_lut, kv_lut):
    """Host-side sharding: returns the per-core input dicts."""
    hs = np.asarray(hidden_states, dtype=np.float32).reshape(T, HID)
    hidT = np.ascontiguousarray(hs.T)
    kvc = np.asarray(kv_cache, dtype=np.float32)
    rc = np.asarray(rope_cache, dtype=np.float32)
    Wq = np.asarray(Wq, dtype=np.float32)
    Wk = np.asarray(Wk, dtype=np.float32)
    Wv = np.asarray(Wv, dtype=np.float32)
    Wo = np.asarray(Wo, dtype=np.float32)
    off = np.asarray(position_offsets, dtype=np.int64)
    dlut = np.asarray(kv_drain_addr_lut, dtype=np.int64)
    klut = np.asarray(kv_lut, dtype=np.int64)

    # Structural facts the device program bakes in; all verified against the
    # actual runtime values.
    assert np.array_equal(klut[:, PAST:], dlut), "drain addrs != tail of kv_lut"
    old = klut[:, :PAST]
    assert not np.isin(old, dlut.reshape(-1)).any(), "old pages clobbered by drain"
    assert np.all(off == PAST), "position offsets != PAST"

    pos = off[:, None] + np.arange(S, dtype=np.int64)[None, :]     # [B,S]
    cos = rc[pos, 0, :].reshape(T, 128).T                           # [128,T]
    sin = rc[pos, 1, :].reshape(T, 128).T
    cs = np.ascontiguousarray(np.stack([cos, sin], axis=0))         # [2,128,T]

    kv_old = kvc[old]                                  # [B, PAST, 2, NKV, HD]
    yy = np.arange(L, dtype=np.int64)[None, :]
    rr = np.arange(128, dtype=np.int64)[:, None]
    bigc = np.ascontiguousarray((yy >= PAST + rr).astype(np.float32))
    in_maps = []
    for c in range(N_CORES):
        koldT = np.ascontiguousarray(kv_old[:, :, 0, c, :].transpose(0, 2, 1))
        vold = np.ascontiguousarray(kv_old[:, :, 1, c, :])
        in_maps.append({
            "hidT": hidT,
            "wq": np.ascontiguousarray(Wq[:, c * 512:(c + 1) * 512]),
            "wk": np.ascontiguousarray(Wk[:, c * HD:(c + 1) * HD]),
            "wv": np.ascontiguousarray(Wv[:, c * HD:(c + 1) * HD]),
            "wo": np.ascontiguousarray(Wo[c * 512:(c + 1) * 512, :]),
            "cs": cs,
            "koldT": koldT,
            "vold": vold,
            "bigc": bigc,
            "ones": np.ones((128, 128), np.float32),
        })
    return in_maps


def kernel(**inputs) -> np.ndarray:
    in_maps = make_in_maps(**inputs)
    nc = _get_program()
    res = run_bass_kernel_spmd(nc, in_maps, core_ids=list(range(N_CORES)))
    out = np.zeros((T, HID), dtype=np.float32)
    for r in res.results:
        out += r["out"]
    return out.reshape(B, S, HID)



# revision 3
# speedup vs baseline: 1.4764x; 1.4764x over previous
"""Trainium2 Bass kernel for a GQA LlamaAttention layer with a LUT-addressed
paged KV cache (B=2, S=1024, HID=4096, NH=32, NKV=8, HD=128, PAST=1024).

Sharding: tensor-parallel over heads across 8 cores. Core c owns query heads
4c..4c+3 and KV head c (column-parallel Wq/Wk/Wv, row-parallel Wo). Each core
produces a full [2048, 4096] partial of out @ Wo in bf16; the host sums the 8
partials in fp32 (row-parallel unshard).

All matmuls in bf16 (1 cycle/row on the PE vs ~2 for fp32 HIGH mode), PSUM
accumulation in fp32. Inputs are converted to bf16 on the host; measured
end-to-end relative error vs the fp32 reference is ~7e-3 (gate 2e-2).

Device kernel (per core):
  phase A: qT/kT/vT = W.T @ hidden.T streamed over 32 k-tiles into wide
           multi-bank PSUM tiles; ACT evacuates PSUM->SBUF with fp32->bf16
           conversion; RoPE applied by DVE as 5 wide ops per token chunk on
           the [head_dim, token] layout; V transposed back to [token,
           head_dim] via PE.
  phase B: per (batch, head): scores computed transposed sT[l, tok] =
           kT_tile.T @ qT (so the P*V matmul needs no transpose), exp via
           ACT with fused 1/sqrt(HD) scale writing bf16 (max-subtraction
           skipped: scores are O(10), safe in fp32 PSUM), causal mask as one
           wide 0/1 multiply, softmax denominator via an all-ones 128x128
           matmul (which also broadcasts the result), fast DVE reciprocal,
           normalization after the V matmul.
  phase C: out_partial[tok, :] += attn.T @ Wo over the 4 local heads; PSUM
           evacuation alternates ACT/DVE to balance engine load.

The LUT structure is used on the host at shard time (values are read from
the actual input arrays and verified): the drain scatter is dead for the
attention output, and old cache rows are host-packed per KV head.
"""

import os
import sys

for _p in ("/root/.axon_site/_ro/trn_rl_repo", "/opt/trn_rl_repo"):
    if os.path.isdir(_p) and _p not in sys.path:
        sys.path.append(_p)

from contextlib import ExitStack

import ml_dtypes
import numpy as np

import concourse.bass as bass
import concourse.mybir as mybir
import concourse.tile as tile
from concourse import bacc
from concourse.bass_utils import run_bass_kernel_spmd

F32 = mybir.dt.float32
BF = mybir.dt.bfloat16
AF = mybir.ActivationFunctionType
NPBF = ml_dtypes.bfloat16

B, S, HID = 2, 1024, 4096
NH, NKV, HD = 32, 8, 128
PAST = 1024
L = PAST + S          # 2048 KV tokens per sequence
T = B * S             # 2048 flattened query tokens
D = HID
HQ = NH // NKV        # 4 query heads per core
NK = D // 128         # 32 contraction tiles for the projections
NJ = L // 128         # 16 l-tiles per batch
SCALE = float(HD) ** -0.5

N_CORES = 8


def _build_program(loop_n: int = 1):
    nc = bacc.Bacc("TRN2", target_bir_lowering=False, debug=False)

    hidT_d = nc.dram_tensor("hidT", [D, T], BF, kind="ExternalInput").ap()
    wq_d = nc.dram_tensor("wq", [D, HQ * HD], BF, kind="ExternalInput").ap()
    wk_d = nc.dram_tensor("wk", [D, HD], BF, kind="ExternalInput").ap()
    wv_d = nc.dram_tensor("wv", [D, HD], BF, kind="ExternalInput").ap()
    wo_d = nc.dram_tensor("wo", [HQ * HD, D], BF, kind="ExternalInput").ap()
    cs_d = nc.dram_tensor("cs", [2, 128, T], BF, kind="ExternalInput").ap()
    koldT_d = nc.dram_tensor("koldT", [B, HD, PAST], BF, kind="ExternalInput").ap()
    vold_d = nc.dram_tensor("vold", [B, PAST, HD], BF, kind="ExternalInput").ap()
    bigc_d = nc.dram_tensor("bigc", [128, L], BF, kind="ExternalInput").ap()
    ones_d = nc.dram_tensor("ones", [128, 128], BF, kind="ExternalInput").ap()
    ident_d = nc.dram_tensor("ident", [128, 128], BF, kind="ExternalInput").ap()
    out_d = nc.dram_tensor("out", [T, D], BF, kind="ExternalOutput").ap()

    with ExitStack() as ctx:
        tc = ctx.enter_context(tile.TileContext(nc))

        def body(_iv=None):
            _emit(nc, tc, hidT_d, wq_d, wk_d, wv_d, wo_d, cs_d, koldT_d,
                  vold_d, bigc_d, ones_d, ident_d, out_d)

        with nc.allow_low_precision("bf16 kernel; fp32 PSUM accumulate"):
            for _ in range(loop_n):
                body()

    nc.compile()
    return nc


def _emit(nc, tc, hidT_d, wq_d, wk_d, wv_d, wo_d, cs_d, koldT_d, vold_d,
          bigc_d, ones_d, ident_d, out_d):
    with ExitStack() as ctx:
        pers = ctx.enter_context(tc.tile_pool(name="pers", bufs=1))

        # ---- persistent SBUF state ----
        # qTall[tc4][hd, h*512 + tok]  (tc4 = global 512-token chunk)
        qTall = [pers.tile([128, HQ * 512], BF, tag=f"qTall{i}",
                           name=f"qTall{i}") for i in range(T // 512)]
        kT = [pers.tile([128, L], BF, tag=f"kT{b}", name=f"kT{b}")
              for b in range(B)]
        # vnb[b][g][l % 128, ((l//128) % 4)*128 + hd]: 512 keys per group
        vnb = [[pers.tile([128, 512], BF, tag=f"vnb{b}_{g}",
                          name=f"vnb{b}_{g}") for g in range(4)]
               for b in range(B)]

        # ---- phase A: projections + RoPE + V transpose ----
        with (
            tc.tile_pool(name="wq_sb", bufs=NK) as wq_pool,
            tc.tile_pool(name="wkv_sb", bufs=NK) as wkv_pool,
            tc.tile_pool(name="hid", bufs=3) as hid_pool,
            tc.tile_pool(name="csbuf", bufs=1) as cs_pool,
            tc.tile_pool(name="ident", bufs=1) as id_pool,
            tc.tile_pool(name="qraw", bufs=1) as qraw_pool,
            tc.tile_pool(name="kvraw", bufs=1) as kvraw_pool,
            tc.tile_pool(name="ropetmp", bufs=1) as rt_pool,
            tc.tile_pool(name="q_ps", bufs=1, space="PSUM") as qps_pool,
            tc.tile_pool(name="kv_ps", bufs=1, space="PSUM") as kvps_pool,
            tc.tile_pool(name="tr_ps", bufs=2, space="PSUM") as trps_pool,
        ):
            cos_sb = cs_pool.tile([128, T], BF, tag="cosf", name="cosf")
            sin_sb = cs_pool.tile([128, T], BF, tag="sinf", name="sinf")
            ident = id_pool.tile([128, 128], BF)
            nc.sync.dma_start(ident[:], ident_d[:])
            wq_sb, wkv_sb = [], []

            def rope6(raw, dest, tok0, nh):
                """RoPE [128, nh*512] raw (heads side by side, same tokens
                tok0..tok0+512) into dest; 5 wide DVE ops. Both inputs of
                every TensorTensor share a base partition (HW constraint)."""
                ts_ = slice(tok0, tok0 + 512)
                def bc(ap):
                    return ap.unsqueeze(1).broadcast_to(
                        [ap.shape[0], nh, 512])
                r3 = lambda ap: ap.rearrange("p (n f) -> p n f", n=nh)
                tr = rt_pool.tile([128, HQ * 512], BF, tag="tr",
                                  name="tr")[:, 0:nh * 512]
                nc.vector.tensor_mul(r3(tr[0:64, :]), r3(raw[64:128, :]),
                                     bc(sin_sb[64:128, ts_]))
                nc.vector.tensor_mul(r3(tr[64:128, :]), r3(raw[0:64, :]),
                                     bc(sin_sb[0:64, ts_]))
                nc.vector.tensor_mul(r3(dest[:, :]), r3(raw[:, :]),
                                     bc(cos_sb[:, ts_]))
                nc.vector.tensor_sub(dest[0:64, :], dest[0:64, :],
                                     tr[0:64, :])
                nc.vector.tensor_add(dest[64:128, :], dest[64:128, :],
                                     tr[64:128, :])

            for tc4 in range(T // 512):
                b = tc4 // 2
                qps = qps_pool.tile([128, HQ * 512], F32, name="qps")
                kvps = kvps_pool.tile([128, 1024], F32, name="kvps")
                for k in range(NK):
                    if tc4 == 0:
                        t = wq_pool.tile([128, HQ * HD], BF, name="wq_sb")
                        nc.sync.dma_start(
                            t[:], wq_d[k * 128:(k + 1) * 128, :])
                        wq_sb.append(t)
                        t = wkv_pool.tile([128, 2 * HD], BF, name="wkv_sb")
                        nc.sync.dma_start(
                            t[:, 0:HD], wk_d[k * 128:(k + 1) * 128, :])
                        nc.sync.dma_start(
                            t[:, HD:2 * HD],
                            wv_d[k * 128:(k + 1) * 128, :])
                        wkv_sb.append(t)
                    ht = hid_pool.tile([128, 512], BF)
                    nc.sync.dma_start(
                        ht[:],
                        hidT_d[k * 128:(k + 1) * 128,
                               tc4 * 512:(tc4 + 1) * 512])
                    st, sp = k == 0, k == NK - 1
                    for h in range(HQ):
                        nc.tensor.matmul(
                            qps[:, h * 512:(h + 1) * 512],
                            wq_sb[k][:, h * 128:(h + 1) * 128],
                            ht[:], start=st, stop=sp)
                    nc.tensor.matmul(kvps[:, 0:512], wkv_sb[k][:, 0:HD],
                                     ht[:], start=st, stop=sp)
                    nc.tensor.matmul(kvps[:, 512:1024],
                                     wkv_sb[k][:, HD:2 * HD],
                                     ht[:], start=st, stop=sp)
                if tc4 == 0:
                    nc.sync.dma_start(cos_sb[:], cs_d[0])
                    nc.sync.dma_start(sin_sb[:], cs_d[1])
                if tc4 == 1:
                    # old KV pages: overlap their DMA with phase-A compute
                    for bb in range(B):
                        nc.sync.dma_start(kT[bb][:, 0:PAST], koldT_d[bb])
                        for g in range(2):
                            src = vold_d[bb, g * 512:(g + 1) * 512, :]
                            nc.sync.dma_start(
                                vnb[bb][g][:].rearrange("p (j h) -> p j h", j=4),
                                src.rearrange("(j p) h -> p j h", p=128))
                # evacuate PSUM on ACT (fp32->bf16 copy), rope on DVE in SBUF
                qraw = qraw_pool.tile([128, HQ * 512], BF, name="qraw")
                nc.scalar.copy(qraw[:], qps[:])
                kvraw = kvraw_pool.tile([128, 1024], BF, name="kvraw")
                nc.scalar.copy(kvraw[:], kvps[:])
                rope6(qraw, qTall[tc4][:], tc4 * 512, HQ)
                koff = PAST + (tc4 % 2) * 512
                rope6(kvraw[:, 0:512], kT[b][:, koff:koff + 512],
                      tc4 * 512, 1)
                # V: 4 PE transposes into one [128, 512] psum, one DVE copy
                g = 2 + tc4 % 2
                tp = trps_pool.tile([128, 512], BF, name="tp")
                for jj in range(4):
                    nc.tensor.transpose(
                        tp[:, jj * 128:(jj + 1) * 128],
                        kvraw[:, 512 + jj * 128:512 + (jj + 1) * 128],
                        ident[:])
                nc.vector.tensor_copy(vnb[b][g][:], tp[:])

        # ---- phase B: attention per (batch, local head) ----
        outT_pool = ctx.enter_context(tc.tile_pool(name="outT_sb", bufs=1))
        # outT[b][hd, h*1024 + tok]: o_proj lhsT layout
        outT = [outT_pool.tile([128, HQ * S], BF, tag=f"outT{b}",
                               name=f"outT{b}") for b in range(B)]
        with (
            tc.tile_pool(name="bigc", bufs=1) as bc_pool,
            tc.tile_pool(name="onesp", bufs=1) as ones_pool,
            tc.tile_pool(name="psT", bufs=14) as psT_pool,
            tc.tile_pool(name="recip", bufs=2) as rec_pool,
            tc.tile_pool(name="sc_ps", bufs=2, space="PSUM") as scps_pool,
            tc.tile_pool(name="av_ps", bufs=1, space="PSUM") as avps_pool,
            tc.tile_pool(name="dn_ps", bufs=1, space="PSUM") as dnps_pool,
        ):
            bigC = bc_pool.tile([128, L], BF)
            nc.sync.dma_start(bigC[:], bigc_d[:])
            ones = ones_pool.tile([128, 128], BF)
            nc.sync.dma_start(ones[:], ones_d[:])

            for b in range(B):
                for h in range(HQ):
                    av = avps_pool.tile([128, 1024], F32, name="av")
                    dn = dnps_pool.tile([128, 1024], F32, name="dn")
                    for t2 in range(2):
                        # tile j is fully causal-masked when
                        # 128*j > PAST + 512*t2 + 511
                        nj_t = 12 if t2 == 0 else NJ
                        qrhs = qTall[b * 2 + t2][:, h * 512:(h + 1) * 512]
                        for jp in range(nj_t // 2):  # j-pairs share one psum
                            sc = scps_pool.tile([128, 1024], F32, name="sc")
                            for jo in range(2):
                                j = jp * 2 + jo
                                nc.tensor.matmul(
                                    sc[:, jo * 512:(jo + 1) * 512],
                                    kT[b][:, j * 128:(j + 1) * 128],
                                    qrhs, start=True, stop=True)
                            pst = psT_pool.tile([128, 2, 512], BF,
                                                name="pst")
                            nc.scalar.activation(
                                pst[:].rearrange("p a b -> p (a b)"),
                                sc[:], AF.Exp, scale=SCALE)
                            if jp >= 4:
                                # causal mask, both tiles of the pair: bigC
                                # cols for tile j=8+j' start at
                                # PAST - 128*j' + t2*512 -> j-dim stride -128
                                moff = PAST - 128 * (jp * 2 - 8) + t2 * 512
                                mask_ap = bass.AP(
                                    tensor=bigC.tensor,
                                    offset=bigC.offset + moff,
                                    ap=[list(bigC.ap[0]), [-128, 2],
                                        [1, 512]])
                                nc.vector.tensor_mul(pst[:], pst[:], mask_ap)
                            # AV + denominator accumulate over the l-tiles
                            # (the all-ones lhsT of dn reduces over l AND
                            # broadcasts the sum to all output partitions)
                            for jo in range(2):
                                j = jp * 2 + jo
                                nc.tensor.matmul(
                                    av[:, t2 * 512:(t2 + 1) * 512],
                                    vnb[b][j // 4][:, (j % 4) * 128:
                                                   (j % 4 + 1) * 128],
                                    pst[:, jo, :],
                                    start=(j == 0), stop=(j == nj_t - 1))
                                nc.tensor.matmul(
                                    dn[:, t2 * 512:(t2 + 1) * 512], ones[:],
                                    pst[:, jo, :],
                                    start=(j == 0), stop=(j == nj_t - 1))
                    rec = rec_pool.tile([128, 1024], F32, name="rec")
                    nc.vector.reciprocal_approx_fast(rec[:], dn[:])
                    nc.vector.tensor_mul(
                        outT[b][:, h * S:(h + 1) * S], av[:], rec[:])

        # ---- phase C: o_proj partial ----
        with (
            tc.tile_pool(name="wo_sb", bufs=HQ) as wo_pool,
            tc.tile_pool(name="ostage", bufs=4) as ost_pool,
            tc.tile_pool(name="op_ps", bufs=4, space="PSUM") as opps_pool,
        ):
            wo_sb = []
            for h in range(HQ):
                t = wo_pool.tile([128, D], BF, name="wo_sb")
                nc.sync.dma_start(t[:], wo_d[h * 128:(h + 1) * 128, :])
                wo_sb.append(t)
            for tt in range(T // 128):
                b, tloc = tt // (S // 128), tt % (S // 128)
                for n8 in range(D // 512):
                    op = opps_pool.tile([128, 512], F32, name="op")
                    for h in range(HQ):
                        nc.tensor.matmul(
                            op[:],
                            outT[b][:, h * S + tloc * 128:
                                    h * S + (tloc + 1) * 128],
                            wo_sb[h][:, n8 * 512:(n8 + 1) * 512],
                            start=(h == 0), stop=(h == HQ - 1))
                    ost = ost_pool.tile([128, 512], BF, name="ost")
                    # alternate evacuation engine to balance ACT/DVE load
                    if n8 % 2 == 0:
                        nc.scalar.copy(ost[:], op[:])
                    else:
                        nc.vector.tensor_copy(ost[:], op[:])
                    nc.sync.dma_start(
                        out_d[tt * 128:(tt + 1) * 128,
                              n8 * 512:(n8 + 1) * 512],
                        ost[:])


_NC_CACHE = {}


def _get_program(loop_n: int = 1):
    if loop_n not in _NC_CACHE:
        _NC_CACHE[loop_n] = _build_program(loop_n)
    return _NC_CACHE[loop_n]


def make_in_maps(hidden_states, kv_cache, rope_cache, Wq, Wk, Wv, Wo,
                 position_offsets, kv_drain_addr_lut, kv_lut):
    """Host-side sharding: returns the per-core input dicts (bf16)."""
    hs = np.asarray(hidden_states, dtype=np.float32).reshape(T, HID)
    hidT = np.ascontiguousarray(hs.T).astype(NPBF)
    kvc = np.asarray(kv_cache, dtype=np.float32)
    rc = np.asarray(rope_cache, dtype=np.float32)
    Wq = np.asarray(Wq, dtype=np.float32).astype(NPBF)
    Wk = np.asarray(Wk, dtype=np.float32).astype(NPBF)
    Wv = np.asarray(Wv, dtype=np.float32).astype(NPBF)
    Wo = np.asarray(Wo, dtype=np.float32).astype(NPBF)
    off = np.asarray(position_offsets, dtype=np.int64)
    dlut = np.asarray(kv_drain_addr_lut, dtype=np.int64)
    klut = np.asarray(kv_lut, dtype=np.int64)

    # Structural facts the device program bakes in; all verified against the
    # actual runtime values.
    assert np.array_equal(klut[:, PAST:], dlut), "drain addrs != tail of kv_lut"
    old = klut[:, :PAST]
    assert not np.isin(old, dlut.reshape(-1)).any(), "old pages clobbered by drain"
    assert np.all(off == PAST), "position offsets != PAST"

    pos = off[:, None] + np.arange(S, dtype=np.int64)[None, :]     # [B,S]
    cos = rc[pos, 0, :].reshape(T, 128).T                           # [128,T]
    sin = rc[pos, 1, :].reshape(T, 128).T
    cs = np.ascontiguousarray(np.stack([cos, sin], axis=0)).astype(NPBF)

    kv_old = kvc[old]                                  # [B, PAST, 2, NKV, HD]
    yy = np.arange(L, dtype=np.int64)[None, :]
    rr = np.arange(128, dtype=np.int64)[:, None]
    bigc = np.ascontiguousarray((yy >= PAST + rr).astype(NPBF))
    in_maps = []
    for c in range(N_CORES):
        koldT = np.ascontiguousarray(
            kv_old[:, :, 0, c, :].transpose(0, 2, 1)).astype(NPBF)
        vold = np.ascontiguousarray(kv_old[:, :, 1, c, :]).astype(NPBF)
        in_maps.append({
            "hidT": hidT,
            "wq": np.ascontiguousarray(Wq[:, c * 512:(c + 1) * 512]),
            "wk": np.ascontiguousarray(Wk[:, c * HD:(c + 1) * HD]),
            "wv": np.ascontiguousarray(Wv[:, c * HD:(c + 1) * HD]),
            "wo": np.ascontiguousarray(Wo[c * 512:(c + 1) * 512, :]),
            "cs": cs,
            "koldT": koldT,
            "vold": vold,
            "bigc": bigc,
            "ones": np.ones((128, 128), NPBF),
            "ident": np.eye(128, dtype=NPBF),
        })
    return in_maps


def kernel(**inputs) -> np.ndarray:
    in_maps = make_in_maps(**inputs)
    nc = _get_program()
    res = run_bass_kernel_spmd(nc, in_maps, core_ids=list(range(N_CORES)))
    out = np.zeros((T, HID), dtype=np.float32)
    for r in res.results:
        out += np.asarray(r["out"], dtype=np.float32)
    return out.reshape(B, S, HID)
